# revision 7
# baseline (speedup 1.0000x reference)
"""Trainium2 Bass kernel for AdaptiveDistillationLoss.

loss = 0.5*mean_i(KL_i) + 0.5*mean_i(CE_i)
  KL_i = sum_j t_ij*(log t_ij - x_ij*rT_i) + lseT_i
  CE_i = lse1_i - x_{i,y_i}
  rT_i = 1/T(conf_i), T piecewise {1.5, 2.0, min(3.7-2c, 3)}
      -> rT = clamp(1/(3.7-2c), 1/3, 0.4) + 0.1*[c>0.6] + (1/6)*[c>0.9]

Everything reduces to three global sums (S12 = sum t*(lt-a), S34 = sum
(lseT+lse1), S5 = sum x_y); each of 8 cores computes per-partition
partials of its shard, host combines:  loss = 0.5*(S12+S34-S5)/B.

Pure data-parallel: no collectives; each core outputs [128, 3] partials.
"""

import sys
import types

import numpy as np

import concourse.bacc as bacc
import concourse.mybir as mybir
import concourse.tile as tile
import concourse.bass_utils as bass_utils
from concourse.bass_utils import run_bass_kernel_spmd


def _install_profile_shims():
    """This image's antenv lacks axon_hooks; register a working NTFF hook
    so run_bass_kernel_spmd(trace=True) can profile. Also make artifact
    upload a local no-op (zero-egress sandbox)."""
    try:
        import antenv.axon_hooks  # noqa: F401
    except ImportError:
        mod = types.ModuleType("antenv.axon_hooks")
        _hook = [None]
        mod.set_axon_ntff_profile_hook = lambda h: _hook.__setitem__(0, h)
        mod.get_axon_ntff_profile_hook = lambda: _hook[0]
        sys.modules["antenv.axon_hooks"] = mod
        import antenv

        antenv.axon_hooks = mod
        try:
            from trn_agent_boot.trn_boot import _ntff_profile_via_ctypes

            mod.set_axon_ntff_profile_hook(
                _ntff_profile_via_ctypes("/opt/axon/libaxon_pjrt.so"))
        except Exception:
            pass
    bass_utils.upload_artifacts = lambda tmpdir: tmpdir


_install_profile_shims()

P = 128
B_FULL = 8388608
NCORES = 8
N_CORE = B_FULL // NCORES  # 1048576 samples per core
FCOLS = N_CORE // P        # 8192 free columns per core

ALU = mybir.AluOpType
ACT = mybir.ActivationFunctionType
F32 = mybir.dt.float32

# module-level knobs (test.py may flip TRACE for profiling)
TRACE = False
F_TILE = 512               # samples per partition per tile
LAST_RESULT = {}           # stash for test.py (exec_time_ns etc.)


def build(nt, f):
    """Build the per-core Bass graph for nt tiles of [P, f] samples."""
    f3 = 3 * f
    nc = bacc.Bacc("TRN2", target_bir_lowering=False)

    x_ext = nc.declare_dram_parameter("logits", [nt, P, f3], F32, isOutput=False)
    t_ext = nc.declare_dram_parameter("soft", [nt, P, f3], F32, isOutput=False)
    c_ext = nc.declare_dram_parameter("conf", [nt, P, f], F32, isOutput=False)
    y_ext = nc.declare_dram_parameter("labels", [nt, P, f], F32, isOutput=False)
    out_ext = nc.declare_dram_parameter("out", [P, 4], F32, isOutput=True)

    with tile.TileContext(nc) as tc:
        with (
            tc.tile_pool(name="io", bufs=3) as io,
            tc.tile_pool(name="wk", bufs=2) as wk,
            tc.tile_pool(name="accp", bufs=1) as accp,
        ):
            acc12 = accp.tile([P, nt], F32, tag="acc12")  # sum t*(lt-a)
            acc34 = accp.tile([P, nt], F32, tag="acc34")  # sum lseT+lse1
            acc5 = accp.tile([P, nt], F32, tag="acc5")    # sum x_y

            for ti in range(nt):
                xin = io.tile([P, f3], F32, tag="xin")
                tin = io.tile([P, f3], F32, tag="tin")
                cin = io.tile([P, f], F32, tag="cin")
                yin = io.tile([P, f], F32, tag="yin")
                nc.sync.dma_start(out=xin[:], in_=x_ext[ti])
                nc.sync.dma_start(out=tin[:], in_=t_ext[ti])
                nc.sync.dma_start(out=cin[:], in_=c_ext[ti])
                nc.sync.dma_start(out=yin[:], in_=y_ext[ti])

                xv = xin[:].rearrange("p (f c) -> p f c", c=3)

                # ---- temperature reciprocal rT ----
                u = wk.tile([P, f], F32, tag="u")
                nc.vector.tensor_scalar(
                    out=u[:], in0=cin[:], scalar1=-2.0, scalar2=3.7,
                    op0=ALU.mult, op1=ALU.add)
                r = wk.tile([P, f], F32, tag="r")
                nc.vector.reciprocal(r[:], u[:])
                rc = wk.tile([P, f], F32, tag="rc")
                nc.vector.tensor_scalar(
                    out=rc[:], in0=r[:], scalar1=1.0 / 3.0, scalar2=0.4,
                    op0=ALU.max, op1=ALU.min)
                m6 = wk.tile([P, f], F32, tag="m6")
                nc.vector.tensor_scalar(
                    out=m6[:], in0=cin[:], scalar1=0.6, scalar2=None,
                    op0=ALU.is_gt)
                m9 = wk.tile([P, f], F32, tag="m9")
                nc.vector.tensor_scalar(
                    out=m9[:], in0=cin[:], scalar1=0.9, scalar2=None,
                    op0=ALU.is_gt)
                s = wk.tile([P, f], F32, tag="s")
                nc.vector.scalar_tensor_tensor(
                    out=s[:], in0=m9[:], scalar=5.0 / 3.0, in1=m6[:],
                    op0=ALU.mult, op1=ALU.add)
                rt = wk.tile([P, f], F32, tag="rt")
                nc.vector.scalar_tensor_tensor(
                    out=rt[:], in0=s[:], scalar=0.1, in1=rc[:],
                    op0=ALU.mult, op1=ALU.add)

                # ---- a = x * rT (broadcast over class dim) ----
                a = wk.tile([P, f3], F32, tag="a")
                av = a[:].rearrange("p (f c) -> p f c", c=3)
                rtb = rt[:].unsqueeze(2).broadcast_to([P, f, 3])
                nc.vector.tensor_mul(out=av, in0=xv, in1=rtb)

                # ---- exponentials, planar output [e0|e1|e2|f0|f1|f2] ----
                ef = wk.tile([P, 6 * f], F32, tag="ef")
                e_pl = ef[:, 0:f3].rearrange("p (j f) -> p f j", j=3)
                f_pl = ef[:, f3:2 * f3].rearrange("p (j f) -> p f j", j=3)
                nc.scalar.activation(e_pl, av, ACT.Exp)
                nc.scalar.activation(f_pl, xv, ACT.Exp)

                # ---- se/sf sums over the 3 planes; ln + accumulate ----
                efv = ef[:].rearrange("p (h j f) -> p h j f", h=2, j=3)
                s01 = wk.tile([P, 2 * f], F32, tag="s01")
                s01v = s01[:].rearrange("p (h f) -> p h f", h=2)
                nc.vector.tensor_add(out=s01v, in0=efv[:, :, 0, :], in1=efv[:, :, 1, :])
                sesf = wk.tile([P, 2 * f], F32, tag="sesf")
                sesfv = sesf[:].rearrange("p (h f) -> p h f", h=2)
                nc.vector.tensor_add(out=sesfv, in0=s01v, in1=efv[:, :, 2, :])
                lnscr = wk.tile([P, 2 * f], F32, tag="lnscr")
                nc.scalar.activation(
                    lnscr[:], sesf[:], ACT.Ln, accum_out=acc34[:, ti:ti + 1])

                # ---- lt = ln(t); z = lt - a; S12 += sum t*z ----
                lt = wk.tile([P, f3], F32, tag="lt")
                nc.scalar.activation(lt[:], tin[:], ACT.Ln)
                z = wk.tile([P, f3], F32, tag="z")
                nc.vector.tensor_sub(out=z[:], in0=lt[:], in1=a[:])
                scr = wk.tile([P, f3], F32, tag="scr")
                nc.vector.scalar_tensor_tensor(
                    out=scr[:], in0=tin[:], scalar=1.0, in1=z[:],
                    op0=ALU.mult, op1=ALU.mult, accum_out=acc12[:, ti:ti + 1])

                # ---- x_y via predicated copies; S5 += sum x_y ----
                m1 = wk.tile([P, f], mybir.dt.uint8, tag="m1")
                nc.vector.tensor_scalar(
                    out=m1[:], in0=yin[:], scalar1=0.5, scalar2=None,
                    op0=ALU.is_ge)
                m2 = wk.tile([P, f], mybir.dt.uint8, tag="m2")
                nc.vector.tensor_scalar(
                    out=m2[:], in0=yin[:], scalar1=1.5, scalar2=None,
                    op0=ALU.is_ge)
                xy = wk.tile([P, f], F32, tag="xy")
                nc.vector.tensor_copy(out=xy[:], in_=xv[:, :, 0])
                nc.vector.copy_predicated(xy[:], m1[:], xv[:, :, 1])
                nc.vector.copy_predicated(xy[:], m2[:], xv[:, :, 2])
                xyscr = wk.tile([P, f], F32, tag="xyscr")
                nc.vector.tensor_scalar(
                    out=xyscr[:], in0=xy[:], scalar1=1.0, scalar2=0.0,
                    op0=ALU.mult, op1=ALU.add, accum_out=acc5[:, ti:ti + 1])

            # ---- final per-partition reduction -> [P, 4] ----
            res = wk.tile([P, 4], F32, tag="res")
            nc.vector.tensor_reduce(
                res[:, 0:1], acc12[:], axis=mybir.AxisListType.X, op=ALU.add)
            nc.vector.tensor_reduce(
                res[:, 1:2], acc34[:], axis=mybir.AxisListType.X, op=ALU.add)
            nc.vector.tensor_reduce(
                res[:, 2:3], acc5[:], axis=mybir.AxisListType.X, op=ALU.add)
            nc.vector.memset(res[:, 3:4], 0.0)
            nc.sync.dma_start(out=out_ext[:], in_=res[:])

    nc.finalize()
    return nc


_BUILD_CACHE = {}


def _get_nc(nt, f):
    key = (nt, f)
    if key not in _BUILD_CACHE:
        _BUILD_CACHE[key] = build(nt, f)
    return _BUILD_CACHE[key]


def kernel(**inputs):
    logits = np.ascontiguousarray(np.asarray(inputs["logits"], dtype=np.float32))
    soft = np.ascontiguousarray(np.asarray(inputs["soft_labels"], dtype=np.float32))
    conf = np.ascontiguousarray(np.asarray(inputs["confidences"], dtype=np.float32))
    labels = np.ascontiguousarray(
        np.asarray(inputs["hard_labels"]).astype(np.float32))

    b = logits.shape[0]
    assert b == B_FULL, f"expected B={B_FULL}, got {b}"
    f = F_TILE
    nt = FCOLS // f
    assert nt * f == FCOLS

    nc = _get_nc(nt, f)

    in_maps = []
    for i in range(NCORES):
        sl = slice(i * N_CORE, (i + 1) * N_CORE)
        in_maps.append({
            "logits": logits[sl].reshape(nt, P, f * 3),
            "soft": soft[sl].reshape(nt, P, f * 3),
            "conf": conf[sl].reshape(nt, P, f),
            "labels": labels[sl].reshape(nt, P, f),
        })

    kres = run_bass_kernel_spmd(
        nc, in_maps, core_ids=list(range(NCORES)), trace=TRACE)
    LAST_RESULT["exec_time_ns"] = kres.exec_time_ns

    s12 = s34 = s5 = 0.0
    for rmap in kres.results:
        o = np.asarray(rmap["out"], dtype=np.float64)
        s12 += o[:, 0].sum()
        s34 += o[:, 1].sum()
        s5 += o[:, 2].sum()
    loss = 0.5 * (s12 + s34 - s5) / float(b)
    return np.float32(loss)
